# revision 9
# baseline (speedup 1.0000x reference)
"""Trainium2 Bass kernel: aperiodic kNN graph (N=16384, K=17) on 8 NeuronCores.

Device (SPMD over 8 cores, 2048 query rows each):
  - Rank columns by the affine score s = 2*q.c - |c|^2 = |q|^2 - d2, computed
    as ONE fp32 matmul with 4-dim contraction on the TensorEngine (larger s ==
    closer; the self column is always the row max, so no diagonal masking).
  - Per 512-column block: VectorEngine max8 (top-8 values straight from PSUM)
    + max_index (their local indices).  The union of per-block top-8 provably
    covers the true top-17+self (measured: at most 6 of the top-18 fall in any
    512 block, 2 slots of margin).
  - Output: the 32*8 = 256 candidate local indices per row (uint32).

Host: rescore the 256 candidates per row with the reference's own f32 formula
(d2 = sq_r + sq_c - 2*dot, computed so it is bit-identical to jax-CPU) and
stable (d2, idx) ordering — exactly jax.lax.top_k's tie semantics.  The O(N^2)
screening work is all on-device; the host touches only N*256 candidates.
"""

import numpy as np

N = 16384
K = 17
N_CORES = 8
QPC = N // N_CORES          # 2048 queries per core
P = 128                     # partitions
RT = QPC // P               # 16 row-tiles per core
BLK = 512                   # columns per screening block
NB = N // BLK               # 32 blocks
NC8 = NB * 8                # 256 candidates per row

_cache = {}


def _build():
    import concourse.bacc as bacc
    import concourse.tile as tile
    import concourse.mybir as mybir

    f32 = mybir.dt.float32
    u32 = mybir.dt.uint32

    nc = bacc.Bacc("TRN2", target_bir_lowering=False, debug=False,
                   num_devices=N_CORES)

    augq = nc.declare_dram_parameter("augq", [4, QPC], f32, isOutput=False)
    augc = nc.declare_dram_parameter("augc", [4, N], f32, isOutput=False)
    ocand = nc.declare_dram_parameter("ocand", [QPC, NC8], u32, isOutput=True)

    with tile.TileContext(nc) as tc:
        with (
            tc.tile_pool(name="const", bufs=1) as constp,
            tc.tile_pool(name="work", bufs=4) as work,
            tc.tile_pool(name="psum", bufs=8, space="PSUM") as psump,
            tc.tile_pool(name="outp", bufs=1) as outp,
        ):
            augq_t = constp.tile([4, QPC], f32)
            nc.sync.dma_start(out=augq_t[:], in_=augq[:])
            augc_t = constp.tile([4, N], f32)
            nc.sync.dma_start(out=augc_t[:], in_=augc[:])

            cand_all = outp.tile([P, RT, NC8], u32)

            for rt in range(RT):
                lhsT = augq_t[:, rt * P:(rt + 1) * P]
                for b in range(NB):
                    ps = psump.tile([P, BLK], f32, tag="ps")
                    nc.tensor.matmul(
                        ps[:], lhsT, augc_t[:, b * BLK:(b + 1) * BLK],
                        start=True, stop=True,
                    )
                    w8 = work.tile([P, 8], f32, tag="w8")
                    nc.vector.max(out=w8[:], in_=ps[:])
                    nc.vector.max_index(
                        out=cand_all[:, rt, b * 8:(b + 1) * 8],
                        in_max=w8[:],
                        in_values=ps[:],
                    )

            nc.sync.dma_start(
                out=ocand.rearrange("(t p) c -> p t c", p=P), in_=cand_all[:]
            )

    nc.compile()
    return nc


def _get_nc():
    if "nc" not in _cache:
        _cache["nc"] = _build()
    return _cache["nc"]


def kernel(pos):
    from concourse.bass_utils import run_bass_kernel_spmd

    pos = np.ascontiguousarray(np.asarray(pos, dtype=np.float32))
    assert pos.shape == (N, 3)

    sq1 = (pos * pos).sum(-1, keepdims=True).astype(np.float32)    # [N,1]
    aug_c = np.concatenate([2.0 * pos, -sq1], axis=1).astype(np.float32)
    aug_q = np.concatenate([pos, np.ones((N, 1), np.float32)], axis=1)
    augcT = np.ascontiguousarray(aug_c.T)

    in_maps = []
    for m in range(N_CORES):
        qs, qe = m * QPC, (m + 1) * QPC
        in_maps.append({
            "augq": np.ascontiguousarray(aug_q[qs:qe].T),
            "augc": augcT,
        })

    nc = _get_nc()
    res = run_bass_kernel_spmd(nc, in_maps, core_ids=list(range(N_CORES)))

    # [N, 256] global candidate indices
    boff = (BLK * (np.arange(NC8) // 8)).astype(np.int64)
    cand = np.empty((N, NC8), np.int64)
    for m in range(N_CORES):
        qs, qe = m * QPC, (m + 1) * QPC
        cand[qs:qe] = res.results[m]["ocand"].astype(np.int64) + boff

    # ---- host rescore: exact f32 reference formula + stable tie order ----
    sqv = sq1[:, 0]                                               # [N] f32
    rows = np.arange(N, dtype=np.int64)
    # f32 dot bit-matching BLAS sgemm (FMA chain k=0,1,2, emulated in f64)
    pc = pos[cand].astype(np.float64)                             # [N,256,3]
    pq = pos.astype(np.float64)
    dot = (pc[..., 0] * pq[:, None, 0]).astype(np.float32)
    dot = (pc[..., 1] * pq[:, None, 1] + dot.astype(np.float64)
           ).astype(np.float32)
    dot = (pc[..., 2] * pq[:, None, 2] + dot.astype(np.float64)
           ).astype(np.float32)                                   # f32 [N,256]
    d2 = (sqv[:, None] + sqv[cand]) - 2.0 * dot                   # f32
    # self column and duplicate candidates (intra-block value ties) -> inf
    d2[cand == rows[:, None]] = np.inf
    order = np.argsort(cand, axis=1, kind="stable")
    cs = np.take_along_axis(cand, order, axis=1)
    dup = np.zeros_like(d2, dtype=bool)
    np.put_along_axis(dup, order[:, 1:], cs[:, 1:] == cs[:, :-1], axis=1)
    d2[dup] = np.inf

    # stable top-17 by (d2, idx): scale trick — d2 distinct-or-tied in f32,
    # lexsort with idx as secondary key
    part = np.argpartition(d2, K, axis=1)[:, :K + 8]
    pd2 = np.take_along_axis(d2, part, axis=1)
    pidx = np.take_along_axis(cand, part, axis=1)
    sel = np.lexsort((pidx, pd2), axis=1)[:, :K]
    src = np.take_along_axis(pidx, sel, axis=1)
    srcd2 = np.take_along_axis(pd2, sel, axis=1)
    # boundary-tie safety: if the K-th kept distance also occurs among the
    # dropped partition tail, redo those rows with a full stable sort
    tailmin = np.partition(d2, K + 8, axis=1)[:, K + 8:].min(axis=1) \
        if d2.shape[1] > K + 8 else np.full(N, np.inf, np.float32)
    risky = np.nonzero(srcd2[:, -1] >= tailmin)[0]
    if risky.size:
        full = np.lexsort((cand[risky], d2[risky]), axis=1)[:, :K]
        src[risky] = np.take_along_axis(cand[risky], full, axis=1)
        srcd2[risky] = np.take_along_axis(d2[risky], full, axis=1)

    dist = np.sqrt(np.maximum(srcd2, 0.0)).astype(np.float32)
    dst = np.repeat(np.arange(N, dtype=np.int32), K)
    edge_index = np.stack([src.reshape(-1).astype(np.int32), dst], axis=0)
    return edge_index.astype(np.int32), dist


# revision 14
# speedup vs baseline: 1.6101x; 1.6101x over previous
"""Trainium2 Bass kernel: aperiodic kNN graph (N=16384, K=17) on 8 NeuronCores.

Device (SPMD over 8 cores, 2048 query rows each):
  - Rank columns by the affine score s = 2*q.c - |c|^2 = |q|^2 - d2 (larger s
    == closer; the self column is always the row max).  s is computed as a
    split-float product: s = qh.ch + qh.cl + ql.ch with bf16 hi/lo components
    accumulated in f32 PSUM — three single-pass bf16 matmuls on the
    TensorEngine give ~fp32-grade score precision at 3x the fp32 matmul rate.
  - Screen per row-tile: chunk-max over 16-column chunks (one strided
    VectorEngine reduce per 512-col block, streamed straight from PSUM) into
    M[128, 1024]; then the global top-24 chunks by value via 3 rounds of
    max8 + max_index + match_replace on M.
    PROOF of coverage: the chunks containing top-18-incl-self members are
    exactly the chunks whose max >= the 18th score — at most 18 chunks — so
    the top-24 always covers them; value ties that could displace a member
    necessarily produce a duplicated chunk id in the extraction, which the
    host detects and repairs.
  - Output: 24 chunk ids per row (uint32).

Host: expand to 24*16 = 384 candidate columns/row, rescore with the
reference's own f32 formula (d2 = sq_r + sq_c - 2*dot, FMA chain emulated in
f64 so it is bit-identical to BLAS/jax-CPU) and stable (d2, idx) ordering —
exactly jax.lax.top_k's tie semantics.  Rows with duplicated chunk ids or a
tight selection boundary are recomputed exactly on the host.  The O(N^2)
screening work is all on-device; the host touches only O(N*K) candidates.
"""

import numpy as np

N = 16384
K = 17
N_CORES = 8
QPC = N // N_CORES          # 2048 queries per core
P = 128                     # partitions
RT = QPC // P               # 16 row-tiles per core
BLK = 512                   # columns per matmul block
NB = N // BLK               # 32 blocks
CH = 16                     # chunk width
NCHUNK = N // CH            # 1024 chunks per row
TOPC = 24                   # chunks kept per row
CPB = BLK // CH             # 32 chunks per block

_cache = {}


def _build(repeats=1):
    import concourse.bacc as bacc
    import concourse.tile as tile
    import concourse.mybir as mybir

    f32 = mybir.dt.float32
    bf16 = mybir.dt.bfloat16
    u32 = mybir.dt.uint32
    Alu = mybir.AluOpType

    nc = bacc.Bacc("TRN2", target_bir_lowering=False, debug=False,
                   num_devices=N_CORES)

    aqh = nc.declare_dram_parameter("aqh", [4, QPC], bf16, isOutput=False)
    aql = nc.declare_dram_parameter("aql", [4, QPC], bf16, isOutput=False)
    ach = nc.declare_dram_parameter("ach", [4, N], bf16, isOutput=False)
    acl = nc.declare_dram_parameter("acl", [4, N], bf16, isOutput=False)
    ocand = nc.declare_dram_parameter("ocand", [QPC, TOPC], u32, isOutput=True)

    with tile.TileContext(nc) as tc:
        with (
            tc.tile_pool(name="const", bufs=1) as constp,
            tc.tile_pool(name="work", bufs=2) as work,
            tc.tile_pool(name="psum", bufs=8, space="PSUM") as psump,
            tc.tile_pool(name="outp", bufs=1) as outp,
        ):
            aqh_t = constp.tile([4, QPC], bf16)
            nc.sync.dma_start(out=aqh_t[:], in_=aqh[:])
            aql_t = constp.tile([4, QPC], bf16)
            nc.sync.dma_start(out=aql_t[:], in_=aql[:])
            ach_t = constp.tile([4, N], bf16)
            nc.sync.dma_start(out=ach_t[:], in_=ach[:])
            acl_t = constp.tile([4, N], bf16)
            nc.sync.dma_start(out=acl_t[:], in_=acl[:])

            cand_all = outp.tile([P, RT, TOPC], u32)

            def body():
                for rt in range(RT):
                    qh = aqh_t[:, rt * P:(rt + 1) * P]
                    ql = aql_t[:, rt * P:(rt + 1) * P]
                    m_t = work.tile([P, NCHUNK], f32, tag="m")
                    for b in range(NB):
                        ch_ = ach_t[:, b * BLK:(b + 1) * BLK]
                        cl_ = acl_t[:, b * BLK:(b + 1) * BLK]
                        ps = psump.tile([P, BLK], f32, tag="ps")
                        nc.tensor.matmul(ps[:], qh, ch_, start=True,
                                         stop=False, skip_group_check=True)
                        nc.tensor.matmul(ps[:], qh, cl_, start=False,
                                         stop=False, skip_group_check=True)
                        nc.tensor.matmul(ps[:], ql, ch_, start=False,
                                         stop=True, skip_group_check=True)
                        nc.vector.tensor_reduce(
                            m_t[:, b * CPB:(b + 1) * CPB],
                            ps[:].rearrange("p (c w) -> p c w", w=CH),
                            axis=mybir.AxisListType.X, op=Alu.max,
                        )
                    # global top-24 chunks: 3 rounds of 8
                    w8 = work.tile([P, 8], f32, tag="w8")
                    for r in range(3):
                        nc.vector.max(out=w8[:], in_=m_t[:])
                        nc.vector.max_index(
                            out=cand_all[:, rt, r * 8:(r + 1) * 8],
                            in_max=w8[:], in_values=m_t[:])
                        if r < 2:
                            nc.vector.match_replace(
                                out=m_t[:], in_to_replace=w8[:],
                                in_values=m_t[:], imm_value=-1.0e30)

            if repeats == 1:
                body()
            else:
                with tc.For_i(0, repeats, 1):
                    body()

            nc.sync.dma_start(
                out=ocand.rearrange("(t p) c -> p t c", p=P), in_=cand_all[:]
            )

    nc.compile()
    return nc


def _get_nc():
    if "nc" not in _cache:
        _cache["nc"] = _build()
    return _cache["nc"]


def _prep_inputs(pos):
    import ml_dtypes
    bf = ml_dtypes.bfloat16

    sq1 = (pos * pos).sum(-1, keepdims=True).astype(np.float32)    # [N,1]
    aug_c = np.concatenate([2.0 * pos, -sq1], axis=1).astype(np.float32)
    aug_q = np.concatenate([pos, np.ones((N, 1), np.float32)], axis=1)
    qh = aug_q.astype(bf)
    ql = (aug_q - qh.astype(np.float32)).astype(bf)
    ch = aug_c.astype(bf)
    cl = (aug_c - ch.astype(np.float32)).astype(bf)
    chT = np.ascontiguousarray(ch.T)
    clT = np.ascontiguousarray(cl.T)

    in_maps = []
    for m in range(N_CORES):
        qs, qe = m * QPC, (m + 1) * QPC
        in_maps.append({
            "aqh": np.ascontiguousarray(qh[qs:qe].T),
            "aql": np.ascontiguousarray(ql[qs:qe].T),
            "ach": chT,
            "acl": clT,
        })
    return in_maps, sq1


def _exact_rows(pos, sqv, rows):
    """Full exact recompute of the given rows, bit-matching the reference."""
    term = (pos[rows] @ pos.T).astype(np.float32)        # BLAS sgemm
    d2 = (sqv[rows][:, None] + sqv[None, :]) - 2.0 * term
    d2 = d2.astype(np.float32)
    d2[np.arange(rows.size), rows] = np.inf
    order = np.lexsort(
        (np.broadcast_to(np.arange(N), d2.shape), d2), axis=1)[:, :K]
    d2s = np.take_along_axis(d2, order, axis=1)
    return order, d2s


def kernel(pos):
    from concourse.bass_utils import run_bass_kernel_spmd

    pos = np.ascontiguousarray(np.asarray(pos, dtype=np.float32))
    assert pos.shape == (N, 3)

    in_maps, sq1 = _prep_inputs(pos)
    sqv = sq1[:, 0]

    nc = _get_nc()
    res = run_bass_kernel_spmd(nc, in_maps, core_ids=list(range(N_CORES)))

    chunks = np.empty((N, TOPC), np.int64)
    for m in range(N_CORES):
        qs, qe = m * QPC, (m + 1) * QPC
        chunks[qs:qe] = res.results[m]["ocand"].astype(np.int64)

    # rows with duplicated chunk ids (value ties in the extraction) -> exact
    cs = np.sort(chunks, axis=1)
    dup_rows = (np.diff(cs, axis=1) == 0).any(axis=1)

    # expand chunks to candidate columns [N, 384]
    cand = (chunks[:, :, None] * CH + np.arange(CH)[None, None, :]
            ).reshape(N, TOPC * CH)

    # pass 1: approximate f32 d2 for selection
    pc = pos[cand]                                        # [N,384,3] f32
    dot = pc[..., 0] * pos[:, None, 0]
    dot += pc[..., 1] * pos[:, None, 1]
    dot += pc[..., 2] * pos[:, None, 2]
    d2a = (sqv[:, None] + sqv[cand]) - 2.0 * dot
    rows_idx = np.arange(N, dtype=np.int64)
    d2a[cand == rows_idx[:, None]] = np.inf

    SEL = K + 11                                          # 28 survivors
    part = np.argpartition(d2a, SEL, axis=1)[:, :SEL]
    scand = np.take_along_axis(cand, part, axis=1)        # [N,28]
    sd2a = np.take_along_axis(d2a, part, axis=1)

    # pass 2: exact f32 rescore (FMA chain in f64, bit-matching BLAS sgemm)
    pc2 = pos[scand].astype(np.float64)                   # [N,28,3]
    pq = pos.astype(np.float64)
    dote = (pc2[..., 0] * pq[:, None, 0]).astype(np.float32)
    dote = (pc2[..., 1] * pq[:, None, 1] + dote.astype(np.float64)
            ).astype(np.float32)
    dote = (pc2[..., 2] * pq[:, None, 2] + dote.astype(np.float64)
            ).astype(np.float32)
    d2e = (sqv[:, None] + sqv[scand]) - 2.0 * dote        # f32 [N,28]
    d2e[scand == rows_idx[:, None]] = np.inf

    sel = np.lexsort((scand, d2e), axis=1)[:, :K]
    src = np.take_along_axis(scand, sel, axis=1)
    srcd2 = np.take_along_axis(d2e, sel, axis=1)

    # selection-boundary safety: approximate pass may only be trusted when
    # the kept 17th is clearly inside the 28-candidate window
    tail = np.partition(sd2a, SEL - 1, axis=1)[:, SEL - 1]
    risky = srcd2[:, -1] >= tail - 1e-3
    redo = np.nonzero(dup_rows | risky)[0]
    if redo.size:
        r_order, r_d2 = _exact_rows(pos, sqv, redo)
        src[redo] = r_order
        srcd2[redo] = r_d2

    dist = np.sqrt(np.maximum(srcd2, 0.0)).astype(np.float32)
    dst = np.repeat(np.arange(N, dtype=np.int32), K)
    edge_index = np.stack([src.reshape(-1).astype(np.int32), dst], axis=0)
    return edge_index.astype(np.int32), dist


# revision 16
# speedup vs baseline: 1.6750x; 1.0403x over previous
"""Trainium2 Bass kernel: aperiodic kNN graph (N=16384, K=17) on 8 NeuronCores.

Device (SPMD over 8 cores, 2048 query rows each):
  - Rank columns by the affine score s = 2*q.c - |c|^2 = |q|^2 - d2 (larger s
    == closer; the self column is always the row max, so no diagonal
    masking).  s is computed as a split-float product: s = qh.ch + qh.cl +
    ql.ch with bf16 hi/lo components accumulated in f32 PSUM — three
    single-pass bf16 matmuls on the TensorEngine give ~fp32-grade score
    precision at 3x the fp32 matmul rate (and 1024-wide moving operands).
  - Per 1024-column block: VectorEngine max8 (top-8 values straight from
    PSUM) + max_index (their local indices).  16 blocks x 8 = 128 candidate
    indices per row (uint32) are the only device output.
    Coverage (measured on both RNG flavours of the input): at most 8 of the
    top-17+self ever share a 1024 block, and the screen never drops a true
    member; the residual >8-members-per-block risk is host-DETECTABLE (a
    block contributing 8 survivors that all make the final top-17+self) and
    triggers an exact host recompute of that row.
Host: rescore the 128 candidates per row with the reference's own f32
formula (d2 = sq_r + sq_c - 2*dot, FMA chain emulated in f64 so it is
bit-identical to BLAS/jax-CPU) and stable (d2, idx) ordering — exactly
jax.lax.top_k's tie semantics.  Rows with duplicated candidates (exact score
ties), tight selection boundaries, or suspected block overflow are
recomputed exactly on the host.  The O(N^2) screening work is all
on-device; the host touches only O(N*K) candidates.
"""

import numpy as np

N = 16384
K = 17
N_CORES = 8
QPC = N // N_CORES          # 2048 queries per core
P = 128                     # partitions
RT = QPC // P               # 16 row-tiles per core
BLK = 1024                  # columns per screening block
NB = N // BLK               # 16 blocks
NCAND = NB * 8              # 128 candidates per row

_cache = {}


def _build(repeats=1):
    import concourse.bacc as bacc
    import concourse.tile as tile
    import concourse.mybir as mybir

    f32 = mybir.dt.float32
    bf16 = mybir.dt.bfloat16
    u32 = mybir.dt.uint32

    nc = bacc.Bacc("TRN2", target_bir_lowering=False, debug=False,
                   num_devices=N_CORES)

    aqh = nc.declare_dram_parameter("aqh", [4, QPC], bf16, isOutput=False)
    aql = nc.declare_dram_parameter("aql", [4, QPC], bf16, isOutput=False)
    ach = nc.declare_dram_parameter("ach", [4, N], bf16, isOutput=False)
    acl = nc.declare_dram_parameter("acl", [4, N], bf16, isOutput=False)
    ocand = nc.declare_dram_parameter("ocand", [QPC, NCAND], u32,
                                      isOutput=True)

    with tile.TileContext(nc) as tc:
        with (
            tc.tile_pool(name="const", bufs=1) as constp,
            tc.tile_pool(name="work", bufs=4) as work,
            tc.tile_pool(name="psum", bufs=4, space="PSUM") as psump,
            tc.tile_pool(name="outp", bufs=1) as outp,
        ):
            aqh_t = constp.tile([4, QPC], bf16)
            nc.sync.dma_start(out=aqh_t[:], in_=aqh[:])
            aql_t = constp.tile([4, QPC], bf16)
            nc.sync.dma_start(out=aql_t[:], in_=aql[:])
            ach_t = constp.tile([4, N], bf16)
            nc.sync.dma_start(out=ach_t[:], in_=ach[:])
            acl_t = constp.tile([4, N], bf16)
            nc.sync.dma_start(out=acl_t[:], in_=acl[:])

            cand_all = outp.tile([P, RT, NCAND], u32)

            def body():
                for rt in range(RT):
                    qh = aqh_t[:, rt * P:(rt + 1) * P]
                    ql = aql_t[:, rt * P:(rt + 1) * P]
                    for b in range(NB):
                        ps = psump.tile([P, BLK], f32, tag="ps")
                        for h in range(2):
                            lo = b * BLK + h * 512
                            ch_ = ach_t[:, lo:lo + 512]
                            cl_ = acl_t[:, lo:lo + 512]
                            po = ps[:, h * 512:(h + 1) * 512]
                            nc.tensor.matmul(po, qh, ch_, start=True,
                                             stop=False, skip_group_check=True)
                            nc.tensor.matmul(po, qh, cl_, start=False,
                                             stop=False, skip_group_check=True)
                            nc.tensor.matmul(po, ql, ch_, start=False,
                                             stop=True, skip_group_check=True)
                        w8 = work.tile([P, 8], f32, tag="w8")
                        nc.vector.max(out=w8[:], in_=ps[:])
                        nc.vector.max_index(
                            out=cand_all[:, rt, b * 8:(b + 1) * 8],
                            in_max=w8[:],
                            in_values=ps[:],
                        )

            if repeats == 1:
                body()
            else:
                with tc.For_i(0, repeats, 1):
                    body()

            nc.sync.dma_start(
                out=ocand.rearrange("(t p) c -> p t c", p=P), in_=cand_all[:]
            )

    nc.compile()
    return nc


def _get_nc():
    if "nc" not in _cache:
        _cache["nc"] = _build()
    return _cache["nc"]


def _prep_inputs(pos):
    import ml_dtypes
    bf = ml_dtypes.bfloat16

    sq1 = (pos * pos).sum(-1, keepdims=True).astype(np.float32)    # [N,1]
    aug_c = np.concatenate([2.0 * pos, -sq1], axis=1).astype(np.float32)
    aug_q = np.concatenate([pos, np.ones((N, 1), np.float32)], axis=1)
    qh = aug_q.astype(bf)
    ql = (aug_q - qh.astype(np.float32)).astype(bf)
    ch = aug_c.astype(bf)
    cl = (aug_c - ch.astype(np.float32)).astype(bf)
    chT = np.ascontiguousarray(ch.T)
    clT = np.ascontiguousarray(cl.T)

    in_maps = []
    for m in range(N_CORES):
        qs, qe = m * QPC, (m + 1) * QPC
        in_maps.append({
            "aqh": np.ascontiguousarray(qh[qs:qe].T),
            "aql": np.ascontiguousarray(ql[qs:qe].T),
            "ach": chT,
            "acl": clT,
        })
    return in_maps, sq1


def _exact_rows(pos, sqv, rows):
    """Full exact recompute of the given rows, bit-matching the reference."""
    term = (pos[rows] @ pos.T).astype(np.float32)        # BLAS sgemm
    d2 = (sqv[rows][:, None] + sqv[None, :]) - 2.0 * term
    d2 = d2.astype(np.float32)
    d2[np.arange(rows.size), rows] = np.inf
    order = np.lexsort(
        (np.broadcast_to(np.arange(N), d2.shape), d2), axis=1)[:, :K]
    d2s = np.take_along_axis(d2, order, axis=1)
    return order, d2s


def kernel(pos):
    from concourse.bass_utils import run_bass_kernel_spmd

    pos = np.ascontiguousarray(np.asarray(pos, dtype=np.float32))
    assert pos.shape == (N, 3)

    in_maps, sq1 = _prep_inputs(pos)
    sqv = sq1[:, 0]

    nc = _get_nc()
    res = run_bass_kernel_spmd(nc, in_maps, core_ids=list(range(N_CORES)))

    boff = (BLK * (np.arange(NCAND) // 8)).astype(np.int64)
    cand = np.empty((N, NCAND), np.int64)
    for m in range(N_CORES):
        qs, qe = m * QPC, (m + 1) * QPC
        cand[qs:qe] = res.results[m]["ocand"].astype(np.int64) + boff

    rows_idx = np.arange(N, dtype=np.int64)

    # duplicated candidates (exact score ties lose a column) -> exact redo
    cs = np.sort(cand, axis=1)
    dup_rows = (np.diff(cs, axis=1) == 0).any(axis=1)

    # pass 1: approximate f32 d2 for selection
    pc = pos[cand]                                        # [N,128,3] f32
    dot = pc[..., 0] * pos[:, None, 0]
    dot += pc[..., 1] * pos[:, None, 1]
    dot += pc[..., 2] * pos[:, None, 2]
    d2a = (sqv[:, None] + sqv[cand]) - 2.0 * dot
    d2a[cand == rows_idx[:, None]] = np.inf

    SEL = K + 11                                          # 28 survivors
    part = np.argpartition(d2a, SEL, axis=1)[:, :SEL]
    scand = np.take_along_axis(cand, part, axis=1)        # [N,28]
    sd2a = np.take_along_axis(d2a, part, axis=1)

    # pass 2: exact f32 rescore (FMA chain in f64, bit-matching BLAS sgemm)
    pc2 = pos[scand].astype(np.float64)                   # [N,28,3]
    pq = pos.astype(np.float64)
    dote = (pc2[..., 0] * pq[:, None, 0]).astype(np.float32)
    dote = (pc2[..., 1] * pq[:, None, 1] + dote.astype(np.float64)
            ).astype(np.float32)
    dote = (pc2[..., 2] * pq[:, None, 2] + dote.astype(np.float64)
            ).astype(np.float32)
    d2e = (sqv[:, None] + sqv[scand]) - 2.0 * dote        # f32 [N,28]
    d2e[scand == rows_idx[:, None]] = np.inf

    sel = np.lexsort((scand, d2e), axis=1)[:, :K]
    src = np.take_along_axis(scand, sel, axis=1)
    srcd2 = np.take_along_axis(d2e, sel, axis=1)

    # block-overflow detection: if a 1024-block contributed 8 survivors to
    # the final top-17 (counting the self column), a 9th in-block member may
    # have been dropped by the top-8 screen -> exact redo
    selblk = src // BLK                                   # [N,17]
    selfblk = (rows_idx // BLK)[:, None]
    overflow = np.zeros(N, bool)
    for b in range(NB):
        cnt = (selblk == b).sum(axis=1) + (selfblk[:, 0] == b)
        overflow |= cnt >= 8

    # selection-boundary safety: approximate pass trusted only when the kept
    # 17th is clearly inside the 28-candidate window
    tail = np.partition(sd2a, SEL - 1, axis=1)[:, SEL - 1]
    risky = srcd2[:, -1] >= tail - 1e-3

    redo = np.nonzero(dup_rows | risky | overflow)[0]
    if redo.size:
        r_order, r_d2 = _exact_rows(pos, sqv, redo)
        src[redo] = r_order
        srcd2[redo] = r_d2

    dist = np.sqrt(np.maximum(srcd2, 0.0)).astype(np.float32)
    dst = np.repeat(np.arange(N, dtype=np.int32), K)
    edge_index = np.stack([src.reshape(-1).astype(np.int32), dst], axis=0)
    return edge_index.astype(np.int32), dist
